# revision 61
# baseline (speedup 1.0000x reference)
"""BitAttention TRN2 kernel: 8-core tensor-parallel (head-split), v3.

Sharding: core c owns heads (2c, 2c+1) = channels [256c, 256c+256) of the
q/k/v projections (column split) and of the output channels of out_proj
(column split).  Attention is fully local to a core; one AllGather of the
(fp16) attention output per (batch, sub-chunk) feeds the out-projection.
The three global reductions in the quantizers (mean|w|, max/min of q/k/v
pre-acts, max/min of out_proj pre-acts) are tiny AllReduces.

v3 changes vs v2 (TimelineSim-driven; the v2 matmul datapath is kept):
  - weight ternarization no longer multiplies by s: for s>0 the compare
    w*s > 0.7*mean|w*s| is scale-invariant, and sign(s) is folded into w
    on the host.  |w| reduces run straight off the DMA'd weights; q/k/v
    ternarize on DVE while o ternarizes on gpsimd in parallel.  Cuts the
    serialized W-phase roughly in half.
  - C2: the pre_k/pre_v reloads are issued before the stats AllReduce and
    the quantization is emitted head-major (q,k on DVE || v on ACT), so
    attention issue only waits for the h=0 slices.
  - den_finish fires one chunk earlier (c%4==1) and the out-projection
    tiles for batch 0 start at c=10 (vs 13), right after batch 0's
    AllGathers are staged.
  - out-proj pre-acts land in an SBUF fp16 arena (no DRAM spill/reload;
    fp16 verified: last layer, no argmax amplification downstream); the
    final quantization runs m=0 on DVE and m=1 on ACT in parallel, in
    half-row chunks so stores overlap.
"""

import numpy as np
import ml_dtypes

DIM = 2048
NCORES = 8
CH = DIM // NCORES          # 256 channels per core
B, S = 2, 2048
T = B * S                   # 4096 tokens
KC = DIM // 128             # 16 contraction chunks
TT = 512                    # token tile
NTT = T // TT
MAGIC = float(1.5 * 2 ** 23)      # fp32 round-to-nearest-even via add/sub
F32MAX = float(np.finfo(np.float32).max)
LOSC = 2048.0                     # 2^11 lo-pass prescale

_cache = {}


def _build(single=False, stop_after=None):
    import concourse.bass as bass  # noqa: F401
    import concourse.mybir as mybir
    import concourse.tile as tile
    from concourse import bacc
    from concourse.bass_isa import ReduceOp

    f32 = mybir.dt.float32
    f16 = mybir.dt.float16
    AX = mybir.AxisListType.X
    OP = mybir.AluOpType
    AF = mybir.ActivationFunctionType

    _ORDER = ["W", "Q", "C2", "S", "G", "O"]

    def _go(ph):
        return stop_after is None or _ORDER.index(ph) <= _ORDER.index(stop_after)

    nc = bacc.Bacc("TRN2", target_bir_lowering=False, debug=False,
                   num_devices=1 if single else NCORES)

    def collective(kind, op, in_ap, out_ap):
        if single:
            if kind == "AllGather":
                # one rank's worth of traffic: the real gather runs on the
                # TOPSP/SDMA collective path, not the kernel's DMA engines,
                # so modelling all 8 ranks here would overstate contention
                rows = in_ap.shape[0]
                nc.sync.dma_start(out_ap[0:rows], in_ap)
            else:
                nc.scalar.dma_start(out_ap, in_ap)
        else:
            nc.gpsimd.collective_compute(kind, op, replica_groups=[list(range(NCORES))],
                                         ins=[in_ap.opt()], outs=[out_ap.opt()])

    def nrecip(pool, out_ap, d_ap, nm, shape=None):
        """out = 1/d with one Newton refinement on top of DVE reciprocal."""
        shape = shape or [d_ap.shape[0], d_ap.shape[-1]]
        g0 = pool.tile(shape, f32, tag=f"nr0_{shape[-1]}", name=f"g0_{nm}")
        t = pool.tile(shape, f32, tag=f"nr1_{shape[-1]}", name=f"t_{nm}")
        u = pool.tile(shape, f32, tag=f"nr2_{shape[-1]}", name=f"u_{nm}")
        nc.vector.reciprocal(g0[:], d_ap)
        nc.vector.tensor_tensor(out=t[:], in0=d_ap, in1=g0[:], op=OP.mult)
        nc.vector.tensor_scalar(out=t[:], in0=t[:], scalar1=1.0, scalar2=None,
                                op0=OP.subtract)
        nc.vector.tensor_tensor(out=u[:], in0=g0[:], in1=t[:], op=OP.mult)
        nc.vector.tensor_tensor(out=out_ap, in0=g0[:], in1=u[:], op=OP.subtract)

    f8e4 = mybir.dt.float8e4
    f8e5 = mybir.dt.float8e5
    DR = mybir.MatmulPerfMode.DoubleRow

    # ---------------- I/O ----------------
    xhi = nc.dram_tensor("xhi", [DIM, T], f16, kind="ExternalInput").ap()
    xl0 = nc.dram_tensor("xl0", [DIM, T], f8e4, kind="ExternalInput").ap()
    xl1 = nc.dram_tensor("xl1", [DIM, T], f8e4, kind="ExternalInput").ap()
    wT = {p: nc.dram_tensor(f"w{p}", [DIM, CH], f32, kind="ExternalInput").ap()
          for p in "qkvo"}
    bias = {p: nc.dram_tensor(f"b{p}", [CH], f32, kind="ExternalInput").ap()
            for p in "qkvo"}
    o_out = nc.dram_tensor("o_out", [CH, T], f32, kind="ExternalOutput").ap()

    xv = {0: xhi.rearrange("(c p) t -> p c t", p=128),
          1: xl0.rearrange("(c p) t -> p c t", p=128),
          2: xl1.rearrange("(c p) t -> p c t", p=128)}
    wTv = {p: wT[p].rearrange("(c p) o -> p c o", p=128) for p in "qkvo"}
    bv = {p: bias[p].rearrange("(m p) -> p m", p=128) for p in "qkvo"}
    o_outv = o_out.rearrange("(m p) t -> p m t", p=128)

    with tile.TileContext(nc) as tc:
        with tc.tile_pool(name="persist", bufs=1) as P, \
             tc.tile_pool(name="dram", bufs=1, space="DRAM") as D:

            # ---- persistent arenas ----
            nqT = P.tile([128, 2, T], f16, name="nqT")        # [d, head, tok]
            nkT = P.tile([128, 2, T], f16, name="nkT")
            n_v = P.tile([128, 2, 2, KC, 128], f16, name="n_v")  # [kp,h,b,kc,ch]
            scal = P.tile([1, 16], f32, name="scal")          # partition-0 scalars
            scalB = P.tile([128, 4], f32, name="scalB")       # broadcast scalars
            stat_q = P.tile([128, 8], f32, name="stat_q")     # qkv+o max/negmin
            wsum = P.tile([128, 4], f32, name="wsum")
            wsum2 = P.tile([128, 4], f32, name="wsum2")
            magicB = P.tile([128, 1], f32, name="magicB")
            nmagicB = P.tile([128, 1], f32, name="nmagicB")

            nc.vector.memset(stat_q[:], -F32MAX)
            nc.vector.memset(magicB[:], MAGIC)
            nc.vector.memset(nmagicB[:], -MAGIC)

            # ---- dram scratch ---- (k m=0 stays in SBUF; only 3 spills)
            pre_d = {p: D.tile([2, 128, T], f32, name=f"pre_{p}") for p in "kv"}
            cc1_in = D.tile([1, 4], f32, name="cc1_in")
            cc1_out = D.tile([1, 4], f32, name="cc1_out", addr_space="Shared")
            cc2_in = D.tile([1, 6], f32, name="cc2_in")
            cc2_out = D.tile([1, 6], f32, name="cc2_out", addr_space="Shared")
            cc3_in = D.tile([1, 2], f32, name="cc3_in")
            cc3_out = D.tile([1, 2], f32, name="cc3_out", addr_space="Shared")
            # contiguous per (batch, sub-chunk): collectives need contiguous APs
            ag_in = D.tile([2, 4, CH, 512], f16, name="ag_in")
            ag_out = [[D.tile([NCORES * CH, 512], f16, name=f"ag_out{b}_{sc}",
                              addr_space="Local" if single else "Shared")
                       for sc in range(4)] for b in range(2)]
            wo_d = D.tile([128, KC, CH], f16, name="wo_d")    # wo parked W->S

            # pre_q and pre_k(m=0) stay SBUF-resident W..C2 (saves serialized
            # DMA spill/reload); scoped outside the weight pool (LIFO)
            with tc.tile_pool(name="preq", bufs=1) as PQ:
              pre_q = PQ.tile([128, 2, T], f32, name="pre_q")
              with tc.tile_pool(name="wter", bufs=1) as WT:
                w_hi = {p: WT.tile([128, KC, CH], f16, name=f"wter_{p}")
                        for p in "qkv"}

                # ============ Phase W: weight ternarization ============
                # s>0 makes the threshold compare scale-free (sign(s) folded
                # into w on the host), so no w*s pass is needed.  q/k/v
                # ternarize on DVE while o runs on gpsimd.
                with tc.tile_pool(name="wstage", bufs=1) as WS:
                    wf = {p: WS.tile([128, KC, CH], f32, name=f"wf_{p}")
                          for p in "qkv"}
                    for p in "qkv":
                        nc.sync.dma_start(wf[p][:], wTv[p])
                    # o streams through a half-matrix buffer (SBUF pressure)
                    wl1o = WS.tile([128, KC], f32, tag="wl1o", name="wl1o")
                    for ohf in range(2):
                        wfoh = WS.tile([128, KC // 2, CH], f32, tag="wfo",
                                       bufs=1, name=f"wfo{ohf}")
                        nc.sync.dma_start(wfoh[:],
                                          wTv["o"][:, ohf * 8:(ohf + 1) * 8, :])
                        nc.vector.tensor_reduce(
                            out=wl1o[:, ohf * 8:(ohf + 1) * 8], in_=wfoh[:],
                            axis=AX, op=OP.add, apply_absolute_value=True)
                    nc.vector.tensor_reduce(
                        out=wsum[:, 3:4], in_=wl1o[:], axis=AX, op=OP.add)
                    for pi, p in enumerate("qkv"):
                        wl1 = WS.tile([128, KC], f32, tag=f"wl1{pi % 2}",
                                      name=f"wl1_{p}")
                        nc.vector.tensor_reduce(
                            out=wl1[:], in_=wf[p][:],
                            axis=AX, op=OP.add, apply_absolute_value=True)
                        nc.vector.tensor_reduce(
                            out=wsum[:, pi:pi + 1], in_=wl1[:],
                            axis=AX, op=OP.add)
                    nc.gpsimd.partition_all_reduce(wsum2[:], wsum[:], channels=128,
                                                   reduce_op=ReduceOp.add)
                    nc.scalar.dma_start(cc1_in[:], wsum2[0:1, 0:4])
                    collective("AllReduce", OP.add, cc1_in[:], cc1_out[:])
                    nc.scalar.dma_start(scal[:, 0:4], cc1_out[:])
                    thr = WS.tile([1, 4], f32, name="thr")
                    nthr = WS.tile([1, 4], f32, name="nthr")
                    nc.vector.tensor_scalar_mul(thr[:], scal[:, 0:4],
                                                0.7 / (DIM * DIM))
                    nc.vector.tensor_scalar_mul(nthr[:], thr[:], -1.0)
                    thr_b = WS.tile([128, 4], f32, name="thr_b")
                    nthr_b = WS.tile([128, 4], f32, name="nthr_b")
                    nc.gpsimd.partition_broadcast(thr_b[:], thr[:])
                    nc.gpsimd.partition_broadcast(nthr_b[:], nthr[:])
                    # ternary = (w > thr) - (w < -thr): the is_lt lands in a
                    # half-size f16 temp and the is_gt+subtract fuse into one
                    # scalar_tensor_tensor.  o on gpsimd (parallel with DVE).
                    for ohf in range(2):
                        osl = slice(ohf * 8, (ohf + 1) * 8)
                        wfoh = WS.tile([128, KC // 2, CH], f32, tag="wfo",
                                       bufs=1, name=f"wfo2_{ohf}")
                        nc.sync.dma_start(wfoh[:], wTv["o"][:, osl, :])
                        lt_o = WS.tile([128, KC // 2, CH], f16, tag="lto",
                                       bufs=1, name=f"lt_o{ohf}")
                        wo_st = WS.tile([128, KC // 2, CH], f16, tag="wost",
                                        bufs=1, name=f"wo_st{ohf}")
                        nc.gpsimd.tensor_scalar(out=lt_o[:], in0=wfoh[:],
                                                scalar1=thr_b[:, 3:4],
                                                scalar2=None, op0=OP.is_gt)
                        nc.gpsimd.tensor_scalar(out=wfoh[:], in0=wfoh[:],
                                                scalar1=nthr_b[:, 3:4],
                                                scalar2=None, op0=OP.is_lt)
                        nc.gpsimd.tensor_tensor(out=wo_st[:], in0=lt_o[:],
                                                in1=wfoh[:], op=OP.subtract)
                        nc.gpsimd.dma_start(wo_d[:, osl, :], wo_st[:])
                    for pi, p in enumerate("qkv"):
                        for hf in range(2):
                            hsl = slice(hf * 8, (hf + 1) * 8)
                            lt = WS.tile([128, KC // 2, CH], f16, tag="lt",
                                         bufs=1, name=f"lt_{p}{hf}")
                            nc.vector.tensor_scalar(
                                out=lt[:], in0=wf[p][:, hsl, :],
                                scalar1=nthr_b[:, pi:pi + 1],
                                scalar2=None, op0=OP.is_lt)
                            nc.vector.scalar_tensor_tensor(
                                out=w_hi[p][:, hsl, :], in0=wf[p][:, hsl, :],
                                scalar=thr_b[:, pi:pi + 1], in1=lt[:],
                                op0=OP.is_gt, op1=OP.subtract)

                # ============ Phase Q: QKV projections ============
                _doQ = _go("Q")
                # allocated after the W staging pool has closed (SBUF re-use)
                pre_k0 = PQ.tile([128, T], f32, name="pre_k0")
                pre_v0 = PQ.tile([128, T], f32, name="pre_v0")
                with tc.tile_pool(name="xstage", bufs=2) as XS, \
                     tc.tile_pool(name="qpsum", bufs=4, space="PSUM") as QP, \
                     tc.tile_pool(name="qout", bufs=2) as QO:
                    def x_load(tt):
                        xs = {}
                        for pas, dt_ in ((0, f16), (1, f8e4), (2, f8e4)):
                            xt = XS.tile([128, KC, TT], dt_, tag=f"x{pas}",
                                         bufs=2 if pas == 0 else 1,
                                         name=f"x{pas}_{tt}")
                            nc.sync.dma_start(
                                xt[:], xv[pas][:, :, tt * TT:(tt + 1) * TT])
                            xs[pas] = xt
                        return xs
                    # tile-0's x loads allocated FIRST so their SBUF region
                    # reuses the early-draining q/k weight staging (the w8a
                    # region lands over the late o/v staging instead)
                    xs0 = x_load(0) if _doQ else None
                    # one 2^-11-scaled ternary set in e5m2 (exact) serves BOTH
                    # fp8 DoubleRow lo passes: the host ships xl1 unscaled
                    # (vs xl0) so the same weight scale applies
                    w8a = {p: XS.tile([128, KC, CH], f8e5, bufs=1,
                                      name=f"w8a_{p}") for p in "qkv"}
                    for p in "qkv":
                        nc.scalar.activation(w8a[p][:], w_hi[p][:],
                                             AF.Identity, bias=0.0,
                                             scale=2.0 ** -11)
                    bsb = QO.tile([128, 3, 2], f32, bufs=1, name="bsb")
                    for pi, p in enumerate("qkv"):
                        nc.sync.dma_start(bsb[:, pi, :], bv[p])
                    for tt in range(NTT if _doQ else 0):
                        xs = xs0 if tt == 0 else x_load(tt)
                        for pi, p in enumerate("qkv"):
                            for m in range(2):
                                msl = slice(m * 128, (m + 1) * 128)
                                ps = QP.tile([128, TT], f32, tag="qp",
                                             name=f"qp{p}{m}{tt}")
                                for kc in range(KC):
                                    nc.tensor.matmul(
                                        ps[:], w_hi[p][:, kc, msl],
                                        xs[0][:, kc, :],
                                        start=(kc == 0), stop=False)
                                # two fp8 DoubleRow lo passes: 256-deep
                                # contraction per instruction, 2x rate
                                for li, xq in enumerate((xs[1], xs[2])):
                                    for kp in range(KC // 2):
                                        nc.tensor.matmul(
                                            ps[:],
                                            w8a[p][:, 2 * kp:2 * kp + 2, msl],
                                            xq[:, 2 * kp:2 * kp + 2, :],
                                            start=False,
                                            stop=(li == 1 and kp == KC // 2 - 1),
                                            perf_mode=DR)
                                tsl = slice(tt * TT, (tt + 1) * TT)
                                if p == "q":
                                    # q pre-acts land straight in SBUF
                                    pre = pre_q[:, m, tsl]
                                elif p == "k" and m == 0:
                                    pre = pre_k0[:, tsl]
                                elif p == "v" and m == 0:
                                    pre = pre_v0[:, tsl]
                                else:
                                    pt = QO.tile([128, TT], f32, tag="pre",
                                                 name=f"pre{p}{m}{tt}")
                                    pre = pt[:]
                                nc.scalar.activation(pre, ps[:], AF.Identity,
                                                     bias=bsb[:, pi, m:m + 1],
                                                     scale=1.0)
                                six = 2 * pi
                                tmx = QO.tile([128, 2], f32, tag="tmx",
                                              name=f"tmx{p}{m}{tt}")
                                nc.vector.tensor_reduce(out=tmx[:, 0:1], in_=pre,
                                                        axis=AX, op=OP.max)
                                nc.vector.tensor_reduce(out=tmx[:, 1:2], in_=pre,
                                                        axis=AX, op=OP.min,
                                                        negate=True)
                                nc.vector.tensor_tensor(out=stat_q[:, six:six + 2],
                                                        in0=stat_q[:, six:six + 2],
                                                        in1=tmx[:], op=OP.max)
                                if m == 1 and p != "q":
                                    nc.scalar.dma_start(
                                        pre_d[p][m, :, tsl], pre)

              _doC2 = _go("C2")
              # ---- global max/min AllReduce + quantize q/k/v ----
              # reloads issued before the AllReduce so they overlap it;
              # quantize is head-major: h=0 first so attention starts early
              stat2 = P.tile([128, 6], f32, name="stat2")
              nc.gpsimd.partition_all_reduce(stat2[:], stat_q[:, 0:6], channels=128,
                                             reduce_op=ReduceOp.max)
              nc.scalar.dma_start(cc2_in[:], stat2[0:1, 0:6])
              with tc.tile_pool(name="qquant", bufs=2) as QQ:
                  collective("AllReduce", OP.max, cc2_in[:], cc2_out[:])
                  nc.scalar.dma_start(scal[:, 4:10], cc2_out[:])
                  # reloads issued after the scal readback so the tiny
                  # critical-path DMA is not queued behind 6us transfers
                  rl = {}
                  for p, m in (("k", 1), ("v", 1)):
                      rl[(p, m)] = QQ.tile([128, T], f32, tag="qst",
                                           bufs=2, name=f"qst{p}{m}")
                      nc.sync.dma_start(rl[(p, m)][:], pre_d[p][m])
                  # one vectorized [1,3] Newton-reciprocal for all 3 scales
                  scl = QQ.tile([1, 3], f32, bufs=1, name="scl")
                  df3 = QQ.tile([1, 3], f32, bufs=1, name="df3")
                  for pi in range(3):
                      nc.vector.tensor_tensor(out=df3[:, pi:pi + 1],
                                              in0=scal[:, 4 + 2 * pi:5 + 2 * pi],
                                              in1=scal[:, 5 + 2 * pi:6 + 2 * pi],
                                              op=OP.add)
                  rcp3 = QQ.tile([1, 3], f32, bufs=1, name="rcp3")
                  nrecip(QQ, rcp3[:], df3[:], "rscl3")
                  nc.vector.tensor_scalar_mul(scl[:], rcp3[:], 255.0)
                  sclB = QQ.tile([128, 3], f32, bufs=1, name="sclB")
                  nc.gpsimd.partition_broadcast(sclB[:], scl[:])

                  def dexp_chain():
                      # Dexp = 1/(scale_q*scale_k*sqrt(128));
                      # scalB: [Dexp, s_v, -Dexp].  Emitted AFTER the q/k m0
                      # quants: first needed by the exp bias ~10us later, so
                      # its 7 small ops must not sit ahead of the quants in
                      # the DVE queue.
                      tmp = QQ.tile([1, 1], f32, bufs=1, name="tmpd")
                      nc.vector.tensor_tensor(out=tmp[:], in0=scl[:, 0:1],
                                              in1=scl[:, 1:2], op=OP.mult)
                      nc.vector.tensor_scalar_mul(tmp[:], tmp[:],
                                                  float(np.sqrt(128.0)))
                      dexp = QQ.tile([1, 1], f32, bufs=1, name="dexp")
                      nrecip(QQ, dexp[:], tmp[:], "rdexp")
                      ndexp = QQ.tile([1, 1], f32, bufs=1, name="ndexp")
                      nc.vector.tensor_scalar_mul(ndexp[:], dexp[:], -1.0)
                      nc.gpsimd.partition_broadcast(scalB[:, 0:1], dexp[:])
                      nc.gpsimd.partition_broadcast(scalB[:, 1:2], scl[:, 2:3])
                      nc.gpsimd.partition_broadcast(scalB[:, 2:3], ndexp[:])
                  # DVE does q m0, k m0 (so attention h=0 starts right away);
                  # the m1 pair runs on the otherwise-idle gpsimd so it never
                  # sits ahead of the attention row-maxes in the DVE queue;
                  # ACT does v m0, v m1
                  def quant_dve(p, m, pi):
                      st = pre_q[:, m, :] if p == "q" else \
                          (pre_k0[:] if m == 0 else rl[(p, m)][:])
                      dst = {"q": nqT, "k": nkT}[p]
                      # half-row (= per-batch) granularity: batch-0 attention
                      # unblocks after the first half of each quant
                      for hf in range(2):
                          hs = slice(hf * S, (hf + 1) * S)
                          t1 = QQ.tile([128, S], f32, tag="qt1",
                                       name=f"qt1{p}{m}{hf}")
                          nc.vector.tensor_scalar(
                              out=t1[:], in0=st[:, hs],
                              scalar1=sclB[:, pi:pi + 1], scalar2=MAGIC,
                              op0=OP.mult, op1=OP.add)
                          nc.vector.tensor_scalar(
                              out=dst[:, m, hs], in0=t1[:], scalar1=MAGIC,
                              scalar2=None, op0=OP.subtract)

                  def quant_v(m):
                      st = pre_v0[:] if m == 0 else rl[("v", m)][:]
                      t1 = QQ.tile([128, T], f32, tag="qt1v", bufs=1,
                                   name=f"qt1v{m}")
                      nvT = QQ.tile([128, T], f16, tag="nvT", bufs=1,
                                    name=f"nvT{m}")
                      nc.scalar.activation(t1[:], st, AF.Identity,
                                           bias=magicB[:, 0:1],
                                           scale=sclB[:, 2:3])
                      nc.scalar.activation(nvT[:], t1[:], AF.Identity,
                                           bias=nmagicB[:, 0:1], scale=1.0)
                      for b in range(2):
                          nc.scalar.dma_start_transpose(
                              n_v[:, m, b], nvT[:, b * S:(b + 1) * S])

                  if _doC2:
                      quant_dve("q", 0, 0)
                      quant_dve("k", 0, 1)
                      dexp_chain()
                      quant_v(0)
                      quant_dve("q", 1, 0)
                      quant_dve("k", 1, 1)
                      quant_v(1)
                  else:
                      dexp_chain()

            # ============ Phase S: attention, single [q,k] pass ============
            # (out-projection tiles interleave into the back half; pre-acts
            # land in the SBUF o_pre arena)
            _doS = _go("S")
            _doO = _go("O")
            with tc.tile_pool(name="opre", bufs=1) as OPool:
              o_pre = OPool.tile([128, 2, T], f16, name="o_pre")
              wo_sb = OPool.tile([128, KC, CH], f16, name="wo_sb")
              bosb = OPool.tile([128, 2], f32, name="bosb")
              den_pad = OPool.tile([128, 128], f16, name="den_pad")
              nc.vector.memset(den_pad[:], 1.0)
              if _doO:
                  nc.sync.dma_start(bosb[:], bv["o"])
                  nc.sync.dma_start(wo_sb[:], wo_d[:])
              with tc.tile_pool(name="sexp", bufs=1) as SE, \
                   tc.tile_pool(name="ssm", bufs=2) as SP, \
                   tc.tile_pool(name="spp", bufs=4, space="PSUM") as PP:

                asts = {}

                def o_load(tt):
                    """stage one gathered token tile (issued a slot early so
                    the 2MB DMA overlaps the previous tile's matmuls)."""
                    b = tt // 4
                    agov = ag_out[b][tt % 4][:].rearrange(
                        "(c p) s -> p c s", p=128)
                    ast = SE.tile([128, KC, TT], f16, tag="ast", bufs=2,
                                  name=f"ast{tt}")
                    nc.sync.dma_start(ast[:], agov)
                    asts[tt] = ast

                def o_half(tt, m):
                    """one out-projection (tile, head) half into o_pre; the
                    halves spread across issue slots as PE gap-filler."""
                    if tt not in asts:
                        o_load(tt)
                    ast = asts[tt] if m == 0 else asts.pop(tt)
                    ps = PP.tile([128, 1024], f32, tag="pss",
                                 name=f"op{m}{tt}")
                    for kc in range(KC):
                        nc.tensor.matmul(
                            ps[:, 0:512],
                            wo_sb[:, kc, m * 128:(m + 1) * 128],
                            ast[:, kc, :],
                            start=(kc == 0), stop=(kc == KC - 1))
                    opre_sl = o_pre[:, m, tt * TT:(tt + 1) * TT]
                    nc.scalar.activation(opre_sl, ps[:, 0:512], AF.Identity,
                                         bias=bosb[:, m:m + 1], scale=1.0)
                    tmx = SP.tile([128, 2], f32, tag="otmx",
                                  name=f"otmx{m}{tt}")
                    nc.vector.tensor_reduce(out=tmx[:, 0:1], in_=opre_sl,
                                            axis=AX, op=OP.max)
                    nc.vector.tensor_reduce(out=tmx[:, 1:2], in_=opre_sl,
                                            axis=AX, op=OP.min, negate=True)
                    nc.vector.tensor_tensor(out=stat_q[:, 6:8],
                                            in0=stat_q[:, 6:8], in1=tmx[:],
                                            op=OP.max)

                def s_scores_qt(h, b, qt, den):
                    """scores + row-max + exp + transpose for one 512-q chunk."""
                    asb = SE.tile([128, 4, S], f16, tag="asb", bufs=2,
                                  name=f"asb{h}{b}{qt}")
                    for qcl in range(4):
                        qc = qt * 4 + qcl
                        q0 = b * S + qc * 128
                        mx2 = SP.tile([128, 2], f32, tag="mx2",
                                      name=f"mx2_{h}{b}{qc}")
                        psl = []
                        for kh in range(2):
                            pss = PP.tile([128, 1024], f32, tag="pss",
                                          name=f"ss{h}{b}{qc}{kh}")
                            for kt in range(2):
                                k0 = b * S + kh * 1024 + kt * 512
                                nc.tensor.matmul(
                                    pss[:, kt * 512:(kt + 1) * 512],
                                    nqT[:, h, q0:q0 + 128],
                                    nkT[:, h, k0:k0 + 512],
                                    start=True, stop=True)
                            nc.vector.tensor_reduce(
                                out=mx2[:, kh:kh + 1], in_=pss[:],
                                axis=AX, op=OP.max)
                            psl.append(pss)
                        nbias = SP.tile([128, 1], f32, tag="nbias",
                                        name=f"nb{h}{b}{qc}")
                        # (-Dexp*mx0) min (-Dexp*mx1) == -Dexp*max(mx0,mx1);
                        # fused + inline on DVE: no cross-engine roundtrip
                        mxs = SP.tile([128, 1], f32, tag="mxs",
                                      name=f"mxs{h}{b}{qc}")
                        nc.vector.tensor_scalar_mul(mxs[:], mx2[:, 1:2],
                                                    scalB[:, 2:3])
                        nc.vector.scalar_tensor_tensor(
                            out=nbias[:], in0=mx2[:, 0:1],
                            scalar=scalB[:, 2:3], in1=mxs[:],
                            op0=OP.mult, op1=OP.min)
                        den2 = SP.tile([128, 2], f32, tag="den2",
                                       name=f"den2{h}{b}{qc}")
                        for kh in range(2):
                            nc.scalar.activation(
                                asb[:, qcl, kh * 1024:(kh + 1) * 1024],
                                psl[kh][:], AF.Exp, bias=nbias[:, 0:1],
                                scale=scalB[:, 0:1],
                                accum_out=den2[:, kh:kh + 1])
                        nc.vector.tensor_tensor(out=den[:, qc:qc + 1],
                                                in0=den2[:, 0:1],
                                                in1=den2[:, 1:2], op=OP.add)
                    atT = SE.tile([128, KC, 4, 128], f16, tag="atT",
                                  bufs=2, name=f"atT{h}{b}{qt}")
                    for qcl in range(4):
                        nc.sync.dma_start_transpose(
                            atT[:, :, qcl, :], asb[:, qcl, :])
                    return atT

                def s_av_qt(h, b, qt, atT, aout_b):
                    """attn @ v for one 512-q chunk (unnormalized)."""
                    po = PP.tile([128, 1024], f32, tag="pss",
                                 name=f"po{h}{b}{qt}")
                    for kc in range(KC):
                        nc.tensor.matmul(po[:, 0:512], n_v[:, h, b, kc, :],
                                         atT[:, kc, :, :],
                                         start=(kc == 0), stop=(kc == KC - 1))
                    qs = qt * 512
                    nc.scalar.copy(aout_b[:, qs:qs + 512], po[:, 0:512])

                def s_den_finish(h, b, den, aout_b):
                    """deferred denominator division, fused with the fp16
                    conversion + allgather staging (sub-chunked)."""
                    dsb = SP.tile([128, 16], f32, tag="dsb", name=f"dsb{h}{b}")
                    nc.vector.tensor_scalar_mul(dsb[:], den[:], scalB[:, 1:2])
                    rden = SP.tile([128, 16], f32, tag="rden",
                                   name=f"rden{h}{b}")
                    nrecip(SP, rden[:], dsb[:], f"rg{h}{b}")
                    # transpose rden via the XBAR (fp16, exact enough at
                    # 2^-11) so the den chain never waits on the PE queue;
                    # the 1.0 padding is memset once (den_pad outlives calls)
                    nc.vector.tensor_copy(den_pad[:, 0:16], rden[:])
                    rdT = SP.tile([128, 128], f16, tag="rdT", bufs=1,
                                  name=f"rdT{h}{b}")
                    nc.scalar.dma_start_transpose(rdT[:], den_pad[:])
                    grow = SP.tile([1, S], f16, tag="grow", bufs=1,
                                   name=f"grow{h}{b}")
                    nc.sync.dma_start(grow[:], rdT[0:16, :])
                    agv = ag_in[b].rearrange("c (m p) s -> p c m s", p=128)
                    # one wide broadcast instead of 4 serial Pool launches
                    gb = SP.tile([128, S], f16, tag="gb", name=f"gb{h}{b}")
                    nc.gpsimd.partition_broadcast(gb[:], grow[:])
                    for sc in range(4):
                        ssl = slice(sc * 512, (sc + 1) * 512)
                        agt = SP.tile([128, 512], f16, tag="agt",
                                      name=f"agt{h}{b}{sc}")
                        nc.vector.tensor_tensor(out=agt[:], in0=aout_b[:, ssl],
                                                in1=gb[:, ssl], op=OP.mult)
                        nc.sync.dma_start(agv[:, sc, h, :], agt[:])
                        if h == 1 and _go("G"):
                            collective("AllGather", OP.bypass,
                                       ag_in[b][sc], ag_out[b][sc][:])

                # globally software-pipelined issue across all 16 chunks:
                # scores(c) before av(c-1); den_finish one chunk after each
                # group's last av (c%4==1); batch-0 out-projection tiles
                # slot in at c=10..13, right after batch-0's allgather.
                chunks = [(b, h, qt)
                          for b in range(2 if _doS else 0)
                          for h in range(2) for qt in range(4)]
                dens = {}
                aouts = {}
                prev = None
                for c in range(len(chunks) + 2 if chunks else 0):
                    if c < len(chunks):
                        b, h, qt = chunks[c]
                        if qt == 0:
                            dens[(h, b)] = SP.tile([128, 16], f32, tag="den",
                                                   bufs=2, name=f"den{h}{b}")
                            aouts[(h, b)] = SE.tile([128, S], f32, tag="aout",
                                                    bufs=2, name=f"aout{h}{b}")
                        atT = s_scores_qt(h, b, qt, dens[(h, b)])
                    if prev is not None:
                        s_av_qt(*prev)
                    prev = ((h, b, qt, atT, aouts[(h, b)])
                            if c < len(chunks) else None)
                    if c >= 5 and c % 4 == 1:
                        g = (c - 5) // 4
                        pb, ph = g // 2, g % 2
                        if (ph, pb) in dens:
                            s_den_finish(ph, pb, dens.pop((ph, pb)),
                                         aouts.pop((ph, pb)))
                    # batch-0 out-proj halves as gap-filler from c=10 on,
                    # with each tile's 2MB load staged one slot ahead
                    if _doO and chunks and 10 <= c <= 17:
                        tt, m = (c - 10) // 2, (c - 10) % 2
                        if m == 0 and tt not in asts:
                            o_load(tt)
                        if m == 1 and tt + 1 <= 3:
                            o_load(tt + 1)
                        o_half(tt, m)
                if chunks:
                    for (ph, pb) in list(dens):
                        s_den_finish(ph, pb, dens.pop((ph, pb)),
                                     aouts.pop((ph, pb)))
                if _doO and chunks:
                    o_load(4)
                    for tt in range(4, 8):
                        if tt + 1 <= 7:
                            o_load(tt + 1)
                        for m in range(2):
                            o_half(tt, m)

              # ============ Phase O tail: final quantization ============
              # (SE/SP/PP pools are closed; only o_pre + OQ are live)
              if _doO and chunks:
                stat3 = P.tile([128, 2], f32, name="stat3")
                nc.gpsimd.partition_all_reduce(stat3[:], stat_q[:, 6:8],
                                               channels=128, reduce_op=ReduceOp.max)
                nc.scalar.dma_start(cc3_in[:], stat3[0:1, 0:2])
                collective("AllReduce", OP.max, cc3_in[:], cc3_out[:])
                nc.scalar.dma_start(scal[:, 10:12], cc3_out[:])
                with tc.tile_pool(name="oquant", bufs=2) as OQ:
                    df = OQ.tile([1, 1], f32, bufs=1, name="odf")
                    nc.vector.tensor_tensor(out=df[:], in0=scal[:, 10:11],
                                            in1=scal[:, 11:12], op=OP.add)
                    oscl = OQ.tile([1, 1], f32, bufs=1, name="oscl")
                    orcp = OQ.tile([1, 1], f32, bufs=1, name="orcp")
                    nrecip(OQ, orcp[:], df[:], "rorcp")
                    nc.vector.tensor_scalar_mul(oscl[:], orcp[:], 255.0)
                    # 1/oscl = df/255 directly (saves a reciprocal chain;
                    # equals the reference dequant to ~1ulp)
                    oinv = OQ.tile([1, 1], f32, bufs=1, name="oinv")
                    nc.vector.tensor_scalar_mul(oinv[:], df[:], 1.0 / 255.0)
                    osclB = OQ.tile([128, 1], f32, bufs=1, name="osclB")
                    oinvB = OQ.tile([128, 1], f32, bufs=1, name="oinvB")
                    nc.gpsimd.partition_broadcast(osclB[:], oscl[:])
                    nc.gpsimd.partition_broadcast(oinvB[:], oinv[:])
                    # 3 chunks on DVE (2 ops each), 1 on ACT (3-op exact
                    # chain), half-row chunks so stores overlap quantize
                    for m in range(2):
                        for hf in range(2):
                            hs = slice(hf * S, (hf + 1) * S)
                            src = o_pre[:, m, hs]
                            t1 = OQ.tile([128, S], f32, tag=f"ot1{m}",
                                         name=f"ot1{m}{hf}")
                            fin = OQ.tile([128, S], f32, tag=f"ofin{m}",
                                          name=f"ofin{m}{hf}")
                            if m == 0:
                                nc.vector.tensor_scalar(
                                    out=t1[:], in0=src,
                                    scalar1=osclB[:], scalar2=MAGIC,
                                    op0=OP.mult, op1=OP.add)
                                nc.vector.tensor_scalar(
                                    out=fin[:], in0=t1[:],
                                    scalar1=MAGIC, scalar2=oinvB[:],
                                    op0=OP.subtract, op1=OP.mult)
                            else:
                                t2 = OQ.tile([128, S], f32, tag="ot2",
                                             name=f"ot2{m}{hf}")
                                nc.scalar.activation(
                                    t1[:], src, AF.Identity,
                                    bias=magicB[:, 0:1],
                                    scale=osclB[:, 0:1])
                                nc.scalar.activation(
                                    t2[:], t1[:], AF.Identity,
                                    bias=nmagicB[:, 0:1], scale=1.0)
                                nc.scalar.activation(
                                    fin[:], t2[:], AF.Identity,
                                    bias=0.0, scale=oinvB[:, 0:1])
                            nc.sync.dma_start(o_outv[:, m, hs], fin[:])

    nc.compile()
    return nc


def kernel(**inputs):
    import concourse.bass_utils as bass_utils

    x = np.asarray(inputs["x"], dtype=np.float32)
    xt = np.ascontiguousarray(x.reshape(T, DIM).T)            # [DIM, T]
    xhi = xt.astype(np.float16)
    f8 = ml_dtypes.float8_e4m3
    r = (xt - xhi.astype(np.float32)) * np.float32(LOSC)      # residual * 2^11
    xl0 = r.astype(f8)
    # second residual UNSCALED so the same 2^-11 weight set serves both lo
    # passes (e4m3 rounding is exponent-shift invariant vs the old *16)
    xl1 = (r - xl0.astype(np.float32)).astype(f8)

    if "nc" not in _cache:
        _cache["nc"] = _build()
    nc = _cache["nc"]

    in_maps = []
    for c in range(NCORES):
        m = {"xhi": xhi, "xl0": xl0, "xl1": xl1}
        for p in "qkvo":
            w = np.asarray(inputs[f"w{p}"], dtype=np.float32)
            # fold sign(s) into w so the on-device ternarization can assume
            # s >= 0 (the threshold compare is then scale-invariant)
            sgn = float(np.sign(np.asarray(inputs[f"s{p}"]).ravel()[0]))
            if sgn == 0.0:
                sgn = 1.0   # s=0: thr=0, ternary of w*0 is 0 either way
            wt = np.ascontiguousarray(w[c * CH:(c + 1) * CH, :].T) * np.float32(sgn)
            m[f"w{p}"] = np.ascontiguousarray(wt)
            m[f"b{p}"] = np.ascontiguousarray(
                np.asarray(inputs[f"b{p}"], dtype=np.float32)[c * CH:(c + 1) * CH])
        in_maps.append(m)

    res = bass_utils.run_bass_kernel_spmd(nc, in_maps, core_ids=list(range(NCORES)))
    full_T = np.concatenate([res.results[c]["o_out"] for c in range(NCORES)], axis=0)
    return np.ascontiguousarray(full_T.T).reshape(B, S, DIM).astype(np.float32)


if __name__ == "__main__":
    d = np.load("/root/problem/inputs_cache.npz")
    out = kernel(**{k: d[k] for k in d.files})
    ref = np.load("/root/problem/ref_out_f32.npy")
    err = np.linalg.norm((out - ref).ravel()) / np.linalg.norm(ref.ravel())
    print("Relative error vs fp32 ref:", err)
